# revision 13
# baseline (speedup 1.0000x reference)
"""Trainium2 Bass kernel for causal multi-head attention (B=2, S=2048, E=1024, H=16).

Sharding: 8 cores = 2 batches x 4 head-groups (4 heads each).
Each core computes its batch's QKV for its 4 heads, causal attention, and a
partial output projection; host sums the 4 group partials per batch + b_out.

V2 changes over the 218us baseline:
- bf16 for all attention-core tensors (qkt/vones/at/mask/pairt/wo/y): halves
  LDWEIGHTS, enables DVE 2x packed modes, and removes the fp32r moving<256
  penalty so causal trims can go to 128-granularity for both heads
- scores+PV trimmed to the exact block-causal minimum (off = 128*m, both
  heads); the unwritten psum window this leaves is only ever read by the exp
  into at-columns that PV never touches
- softmax reciprocal moved back to the DVE (the scalar Ln+exp staging chain
  was gating the next phase's PE work through psum-buffer reuse at every
  hp/chunk boundary, each stall also resetting the PE p-state to 1.2GHz);
  recips+bcast+muls+adds are all deferred units woven into the next phase
- startup: x chunk-0 DMA'd in token-tile pieces, v-units run first (each
  needs only its piece), qk1/qk3 pushed into the hp0 filler weave, and bf16
  warm-up matmuls ramp the PE p-state while HBM streams.
"""
import sys

sys.path.insert(0, "/opt/trn_rl_repo")

from contextlib import ExitStack

import numpy as np

import concourse.bass as bass
import concourse.tile as tile
from concourse import bacc, mybir
from concourse.bass_utils import run_bass_kernel_spmd

dt = mybir.dt

B, S, E, H = 2, 2048, 1024, 16
HD = 64                     # head dim
HPC = 4                     # heads per core
NC = 8                      # cores
KE = E // 128               # 8 contraction k-tiles for projections
NT = S // 128               # 16 token tiles
NCH = S // 512              # 4 token chunks
FQK = 512                   # q+k features per core (4 heads * 64 * 2)
FV = 256                    # v features per core


class _Bacc(bacc.Bacc):
    """Pin the activation-table fixpoint to the combined ln+exp table so the
    Exp mix never thrashes ACT_TABLE_LOADs."""

    def insert_act_table_loads(self):
        import bass_rust as _bass_rust
        from concourse.hw_specs import get_activation_tables

        has_activation = any(
            isinstance(i, mybir.InstActivation)
            for b in self.main_func.blocks
            for i in b.instructions
        )
        if not has_activation:
            return
        tables = [
            (name, funcs if name == "natural_log_exp_and_others" else set())
            for name, funcs in get_activation_tables(self.m.arch).items()
        ]
        _bass_rust.insert_act_table_loads(self, tables)


def _build_program():
    nc = _Bacc("TRN2", target_bir_lowering=False, debug=False, num_devices=NC)

    xT_d = nc.dram_tensor("xT", [E, S], dt.bfloat16, kind="ExternalInput")
    wqkT_d = nc.dram_tensor("wqkT", [E, FQK], dt.bfloat16, kind="ExternalInput")
    wvT_d = nc.dram_tensor("wvT", [E, FV], dt.bfloat16, kind="ExternalInput")
    bqk_d = nc.dram_tensor("bqk", [FQK], dt.float32, kind="ExternalInput")
    bv_d = nc.dram_tensor("bv", [FV], dt.float32, kind="ExternalInput")
    wo_d = nc.dram_tensor("wo", [FV, E], dt.bfloat16, kind="ExternalInput")
    mask_d = nc.dram_tensor("trimask", [128, 128], dt.bfloat16, kind="ExternalInput")
    sel_d = nc.dram_tensor("sel", [128, 128], dt.float32r, kind="ExternalInput")
    y_d = nc.dram_tensor("y", [S, E], dt.bfloat16, kind="ExternalOutput")

    with TileKernel(nc) as tk:
        tk.build(xT_d, wqkT_d, wvT_d, bqk_d, bv_d, wo_d, mask_d, sel_d, y_d)
    nc.compile()
    return nc


class TileKernel:
    def __init__(self, nc):
        self.nc = nc
        self.ctx = ExitStack()
        self.tc_cm = tile.TileContext(nc)

    def __enter__(self):
        self.tc = self.tc_cm.__enter__()
        return self

    def __exit__(self, *a):
        self.ctx.close()
        return self.tc_cm.__exit__(*a)

    def build(self, xT_d, wqkT_d, wvT_d, bqk_d, bv_d, wo_d, mask_d, sel_d, y_d):
        nc, tc, ctx = self.nc, self.tc, self.ctx
        pool = lambda name, bufs, **kw: ctx.enter_context(
            tc.tile_pool(name=name, bufs=bufs, **kw)
        )

        const_p = pool("const", 1)
        xs_p = pool("xs", 2)
        qkt_p = pool("qkt", 1)
        vones_p = pool("vones", 1)
        attn_p = pool("attn", 8)
        pair_p = pool("pair", 1)
        small_p = pool("small", 2)
        y_p = pool("y", 4)
        # PSUM: ps (2 bufs x [128,1024] = 4 banks) + po (4 tags x 1 bank) = 8
        ps_p = pool("ps", 2, space="PSUM")
        po_p = pool("po", 1, space="PSUM")
        self.small_p = small_p
        self.ps_p = ps_p

        # ---- warm the exp activation table before any DMA lands ----
        warm = const_p.tile([1, 16], dt.float32, tag="warm")
        nc.vector.memset(warm[:], 0.0)
        nc.scalar.activation(warm[:], warm[:], mybir.ActivationFunctionType.Exp)

        # bf16 warm-up tiles for PE p-state ramp: no DMA dependency, so the
        # dummies start immediately and run at 1 cycle/row at any p-state
        dum = const_p.tile([128, 512], dt.bfloat16, tag="dum")
        nc.vector.memset(dum[:], 0.0)

        # selector constant (tiny, used by the normalize broadcast)
        sel_sb = const_p.tile([128, 128], dt.float32r, tag="sel")
        nc.sync.dma_start(sel_sb[:], sel_d[:])
        self.sel_sb = sel_sb

        # ---- weights/x. chunk-0 x arrives in 4 token-tile pieces on the
        # scalar queue so the v-units can start after the first ~0.5MB;
        # wqk streams on the sync queue in parallel ----
        wqk_big = const_p.tile([128, KE * FQK], dt.bfloat16, tag="wqk")
        for ke in range(KE):
            nc.sync.dma_start(
                wqk_big[:, FQK * ke : FQK * (ke + 1)],
                wqkT_d[128 * ke : 128 * (ke + 1), :],
            )
        wqk_r = [wqk_big[:, FQK * ke : FQK * (ke + 1)] for ke in range(KE)]

        xs0 = xs_p.tile([128, KE * 512], dt.bfloat16, tag="xs", name="xs0")
        for ke in range(KE):
            nc.scalar.dma_start(
                xs0[:, 512 * ke : 512 * (ke + 1)],
                xT_d[128 * ke : 128 * (ke + 1), 0:512],
            )
        self.xs0 = xs0

        # v weights on the gpsimd queue (v-units need them first)
        wv_big = const_p.tile([128, KE * FV], dt.bfloat16, tag="wv")
        for ke in range(KE):
            nc.gpsimd.dma_start(
                wv_big[:, FV * ke : FV * (ke + 1)],
                wvT_d[128 * ke : 128 * (ke + 1), :],
            )
        wv_r = [wv_big[:, FV * ke : FV * (ke + 1)] for ke in range(KE)]

        bqk_sb = const_p.tile([128, 4], dt.float32, tag="bqk")
        nc.sync.dma_start(bqk_sb[:], bqk_d[:].rearrange("(f p) -> p f", p=128))
        bv_sb = const_p.tile([128, 2], dt.float32, tag="bv")
        nc.sync.dma_start(bv_sb[:], bv_d[:].rearrange("(f p) -> p f", p=128))
        mask_sb = const_p.tile([128, 128], dt.bfloat16, tag="mask")
        nc.sync.dma_start(mask_sb[:], mask_d[:])

        wo_big = const_p.tile([128, 2 * E], dt.bfloat16, tag="wo")
        for kt in range(2):
            nc.gpsimd.dma_start(
                wo_big[:, E * kt : E * (kt + 1)],
                wo_d[128 * kt : 128 * (kt + 1), :],
            )
        wo_r = [wo_big[:, E * kt : E * (kt + 1)] for kt in range(2)]

        # persistent reciprocal tile (f32r: feeds the selector matmul).
        # ones-init once; DVE reciprocals rewrite only rows 0/32/64/96
        onef = const_p.tile([128, 512], dt.float32, tag="onef")
        nc.vector.memset(onef[:], 1.0)
        self.rc = const_p.tile([128, 512], dt.float32r, tag="rc")
        nc.vector.tensor_copy(self.rc[:], onef[:])

        # ---- persistent activations ----
        # qkt tiles: 0: q heads 0,1 | 1: q heads 2,3 | 2: k heads 0,1 | 3: k heads 2,3
        qkt = [qkt_p.tile([128, S], dt.bfloat16, tag=f"qkt{f}", name=f"qkt{f}") for f in range(4)]
        # vones[t]: [v h0 |1| v h1 |1| v h2 |1| v h3 |1] for token tile t
        vones = [vones_p.tile([128, 4 * 65], dt.bfloat16, tag=f"v{t}", name=f"v{t}") for t in range(NT)]
        # ones columns are written once here; v_unit only writes the v parts
        ones_sb = const_p.tile([128, 1, 1], dt.bfloat16, tag="ones")
        nc.vector.memset(ones_sb[:], 1.0)
        for t in range(NT):
            v3 = vones[t][:].rearrange("p (g d) -> p g d", d=65)
            nc.vector.tensor_copy(v3[:, :, 64:65], ones_sb[:].to_broadcast((128, 4, 1)))
        # pair tiles: final normalized attn output, [head dims x 2, S]
        pairt = [pair_p.tile([128, S], dt.bfloat16, tag=f"pair{hp}", name=f"pair{hp}") for hp in range(2)]

        env = dict(
            xT_d=xT_d, wqk_r=wqk_r, wv_r=wv_r, bqk_sb=bqk_sb,
            xs_p=xs_p, ps_p=ps_p, po_p=po_p, attn_p=attn_p, small_p=small_p,
            qkt=qkt, vones=vones,
            pairt=pairt, bv_sb=bv_sb, mask_sb=mask_sb, wo_r=wo_r,
            y_p=y_p, y_d=y_d,
        )
        # startup: while x0/wqk stream from HBM, run bf16 dummy matmuls so
        # the PE p-state ramps; then the v-units (piece-sized deps) and the
        # two qk units attention-hp0 needs. qk1/qk3 weave into hp0's loop.
        env[("xs", 0)] = xs0
        pwarm = ps_p.tile([128, 1024], dt.float32, tag="ps", name="pwarm")
        for _ in range(12):
            nc.tensor.matmul(pwarm[:, 0:512], dum[:, 0:128], dum[:],
                             start=True, stop=True)
        qkv0 = self.qkv_units(0, env)
        def dummy():
            nc.tensor.matmul(pwarm[:, 0:512], dum[:, 0:128], dum[:],
                             start=True, stop=True)
        for u in qkv0[4:]:      # v units t4=0..3, dummies pad DMA waits
            u()
            dummy()
        qkv0[0]()               # qk f=0 (q heads 0,1)
        dummy()
        qkv0[2]()               # qk f=2 (k heads 0,1)
        dummy()
        startup_rest = [qkv0[1], qkv0[3]]
        # filler schedule: qkv(c+1) weaves into chunk c; out-projections are
        # pushed late so chunk 3 (the longest, scalar-bound) keeps PE work:
        # oproj(0)->c1/hp1, oproj(1)->c3/hp0, oproj(2)->c3/hp1, oproj(3)->tail
        deferred = []  # normalize units of the previous chunk's rollout
        for c in range(NCH):
            if c + 1 < NCH:
                self.x_dma(c + 1, env)
                qkv = self.qkv_units(c + 1, env)
                qk_u, v_u = qkv[:4], qkv[4:]
            else:
                qk_u, v_u = [], []
            f1 = list(v_u)
            if c == 3:
                other = self.oproj_units(1, env)
                f1 += self.oproj_units(0, env) + self.oproj_units(2, env)
            else:
                other = startup_rest + qk_u
                startup_rest = []
            deferred = self.attention_chunk(c, env, deferred, other, f1)
        for u in deferred:
            u()
        for u in self.oproj_units(NCH - 1, env, copy_eng="alt"):
            u()

    # ------------------------------------------------------------------
    def x_dma(self, c, env):
        nc = self.nc
        cs = slice(512 * c, 512 * (c + 1))
        if c == 0:
            return
        xs = env["xs_p"].tile([128, KE * 512], dt.bfloat16, tag="xs", name=f"xs{c}")
        # one descriptor-gen on the sync sequencer instead of eight
        nc.sync.dma_start(
            xs[:].rearrange("p (ke f) -> p ke f", f=512),
            env["xT_d"][:, cs].rearrange("(ke p) f -> p ke f", p=128),
        )
        env[("xs", c)] = xs

    # ------------------------------------------------------------------
    def qkv_units(self, c, env):
        nc = self.nc
        cs = slice(512 * c, 512 * (c + 1))
        wqk_r, wv_r = env["wqk_r"], env["wv_r"]
        qkt, vones = env["qkt"], env["vones"]
        bqk_sb = env["bqk_sb"]
        ps_p = env["ps_p"]
        xs = env[("xs", c)]
        xr = [xs[:, 512 * ke : 512 * (ke + 1)] for ke in range(KE)]

        def qk_unit(f):
            pq = ps_p.tile([128, 1024], dt.float32, tag="ps", name="pq")
            for ke in range(KE):
                nc.tensor.matmul(
                    pq[:, 0:512], wqk_r[ke][:, 128 * f : 128 * (f + 1)], xr[ke],
                    start=(ke == 0), stop=(ke == KE - 1),
                )
            nc.vector.tensor_scalar_add(qkt[f][:, cs], pq[:, 0:512], bqk_sb[:, f : f + 1])

        def v_unit(t4):
            t = 4 * c + t4
            pv = ps_p.tile([128, 1024], dt.float32, tag="ps", name="pv")
            for ke in range(KE):
                nc.tensor.matmul(
                    pv[:, 0:FV],
                    xr[ke][:, 128 * t4 : 128 * (t4 + 1)], wv_r[ke],
                    start=(ke == 0), stop=(ke == KE - 1),
                )
            v3 = vones[t][:].rearrange("p (g d) -> p g d", d=65)
            nc.vector.tensor_copy(
                v3[:, :, 0:64],
                pv[:, 0:FV].rearrange("p (g d) -> p g d", d=64),
            )

        units = []
        for f in range(4):
            units.append(lambda f=f: qk_unit(f))
        for t4 in range(4):
            units.append(lambda t4=t4: v_unit(t4))
        return units

    # ------------------------------------------------------------------
    def oproj_units(self, c, env, copy_eng="vector"):
        nc = self.nc
        pairt, wo_r, ps_p, y_p, y_d = (
            env["pairt"], env["wo_r"], env["ps_p"], env["y_p"], env["y_d"])
        units = []

        def unit(t4):
            t = 4 * c + t4
            ysb = y_p.tile([128, E], dt.bfloat16, tag="y", name="ysb")
            py = ps_p.tile([128, 1024], dt.float32, tag="ps", name="py")
            for o in range(2):
                for kt in range(2):
                    nc.tensor.matmul(
                        py[:, 512 * o : 512 * (o + 1)],
                        pairt[kt][:, 128 * t : 128 * (t + 1)],
                        wo_r[kt][:, 512 * o : 512 * (o + 1)],
                        start=(kt == 0), stop=(kt == 1),
                    )
            ce = copy_eng if copy_eng != "alt" else ("scalar" if t4 % 2 else "vector")
            if ce == "scalar":
                nc.scalar.activation(ysb[:], py[:], mybir.ActivationFunctionType.Copy)
            else:
                nc.vector.tensor_copy(ysb[:], py[:])
            eng = nc.gpsimd if t % 2 == 0 else nc.sync
            eng.dma_start(y_d[128 * t : 128 * (t + 1), :], ysb[:])

        for t4 in range(4):
            units.append(lambda t4=t4: unit(t4))
        return units

    # ------------------------------------------------------------------
    def attention_chunk(self, c, env, pre, fillers0, f1_extra):
        """Attention for both head pairs of chunk c. Returns the deferred
        normalize units of the chunk's rollout for the caller to weave into
        the next phase. `pre` holds the previous chunk's normalize units:
        they read the po banks this chunk's first start=True PV reclaims,
        and the bcast matmul sits behind that PV in the PE FIFO, so every
        pre unit must be emitted before the first PV pop (j == PV_DEPTH)."""
        nc = self.nc
        qkt, vones = env["qkt"], env["vones"]
        ps_p, po_p, attn_p = env["ps_p"], env["po_p"], env["attn_p"]
        mask_sb = env["mask_sb"]
        nj = 4 * c + 4
        # po[2*hp + h_idx]: [65, 512] accumulator per head
        po = [po_p.tile([65, 512], dt.float32, tag=f"po{i}", name=f"po{i}")
              for i in range(4)]

        def emit_pv(hp, j, off, at):
            for h_idx in range(2):
                slot = 2 * hp + h_idx
                nc.tensor.matmul(
                    po[slot][:, off:512],
                    vones[j][:, 65 * slot : 65 * slot + 65],
                    at[:, 512 * h_idx + off : 512 * (h_idx + 1)],
                    start=(j == 0), stop=(j == nj - 1),
                    skip_group_check=True,
                )

        last = c == NCH - 1
        PV_DEPTH = 6
        part2_hp0 = None
        for hp in range(2):
            if hp == 0:
                p_units, fillers = list(pre), fillers0
            else:
                p_units, fillers = (part2_hp0 or []), f1_extra
            pre_start = min(2, nj - 1)
            slots_pre = max(1, min(PV_DEPTH, nj) - pre_start)
            quota = (len(p_units) + slots_pre - 1) // slots_pre if p_units else 0
            nfill = len(fillers)
            # reserve a couple of PE-heavy fillers for after the drain so
            # the PE stays warm across the chunk boundary while the last
            # exps retire
            nfill_w = max(0, nfill - 2) if hp == 1 else nfill
            emitted = 0
            pending = []
            for j in range(nj):
                ps = ps_p.tile([128, 1024], dt.float32, tag="ps", name="ps")
                at = attn_p.tile([128, 1024], dt.bfloat16, tag="attn", name="at")
                m = j - 4 * c
                off = 128 * m if 1 <= m <= 3 else 0
                for h_idx in range(2):
                    # bf16 runs 1 cycle/row at any moving size, so both
                    # heads trim to the exact causal boundary. The psum
                    # window [512, 512+off) this leaves unwritten holds
                    # stale-but-finite data; its exp lands in at-columns
                    # PV never reads.
                    r0 = 64 * h_idx
                    nc.tensor.matmul(
                        ps[:, 512 * h_idx + off : 512 * (h_idx + 1)],
                        qkt[2 + hp][r0 : r0 + 64, 128 * j : 128 * (j + 1)],
                        qkt[hp][r0 : r0 + 64, 512 * c + off : 512 * (c + 1)],
                        start=True, stop=True,
                    )
                nc.scalar.activation(
                    at[:, off:1024], ps[:, off:1024],
                    mybir.ActivationFunctionType.Exp)
                if m >= 0:
                    # causal mask: zero the upper triangle of the diagonal
                    # block, both heads in one strided op (bf16: DVE 2x)
                    av = at[:].rearrange("p (h q) -> p h q", h=2)
                    dg = av[:, :, 128 * m : 128 * (m + 1)]
                    mv = (mask_sb[:].rearrange("a (o n) -> a o n", o=1)
                          .to_broadcast((128, 2, 128)))
                    nc.vector.tensor_mul(dg, dg, mv)
                pending.append((j, off, at))
                if j >= pre_start:
                    for _ in range(quota):
                        if p_units:
                            p_units.pop(0)()
                if len(pending) > PV_DEPTH:
                    emit_pv(hp, *pending.pop(0))
                while emitted < nfill_w and emitted * nj < (j + 1) * nfill:
                    fillers[emitted]()
                    emitted += 1
            for u in p_units:
                u()
            for p in pending:
                emit_pv(hp, *p)
            while emitted < nfill:
                fillers[emitted]()
                emitted += 1
            if last and hp == 0:
                part2_hp0 = self.rollout(c, env, po, [0])
        if last:
            return self.rollout(c, env, po, [1])
        return self.rollout(c, env, po, [0, 1])

    # ------------------------------------------------------------------
    def rollout(self, c, env, po, hps):
        """Normalize head pairs `hps` of chunk c: pairt[hp][:,chunk] =
        po_v / denominator + bv. All units (DVE reciprocals of the psum
        denominator rows, the selector-broadcast PE outer product, the
        multiplies and the bias-add) are returned as deferred units the
        caller weaves into a later phase, so nothing gates the PE at the
        boundary."""
        nc = self.nc
        small_p, pairt, bv_sb = self.small_p, env["pairt"], env["bv_sb"]
        rc = self.rc

        def stage_ln(hp, h_idx):
            # ln(denominator) staged on the scalar engine; 1/d is then a
            # single Exp(-x). Both live in the pinned ln+exp activation
            # table, so no ACT_TABLE_LOAD thrash — a direct Reciprocal
            # would swap tables (1.3us) twice per chunk, and the DVE
            # reciprocal is an iterative op (~3.4us per [1,512] row).
            ln = mybir.ActivationFunctionType.Ln
            nc.scalar.activation(
                rc[64 * hp + 32 * h_idx : 64 * hp + 32 * h_idx + 1, :],
                po[2 * hp + h_idx][64:65, :], ln)

        def rcexp():
            lo, hi = 64 * hps[0], 64 * hps[-1] + 33
            nc.scalar.activation(
                rc[lo:hi, :], rc[lo:hi, :], mybir.ActivationFunctionType.Exp,
                scale=-1.0)

        bchs = {}

        def bcast_stage():
            bch_ps = self.ps_p.tile([128, 1024], dt.float32, tag="ps", name="bch_ps")
            for hp in hps:
                nc.tensor.matmul(
                    bch_ps[:, 512 * hp : 512 * (hp + 1)],
                    self.sel_sb[64 * hp : 64 * hp + 64, :],
                    rc[64 * hp : 64 * hp + 64, :],
                    start=True, stop=True,
                )
            bchs["t"] = small_p.tile([128, 1024], dt.float32, tag="bch", name="bchs")
            if len(hps) == 2:
                nc.vector.tensor_copy(bchs["t"][:], bch_ps[:])
            else:
                hp = hps[0]
                nc.vector.tensor_copy(
                    bchs["t"][:, 512 * hp : 512 * (hp + 1)],
                    bch_ps[:, 512 * hp : 512 * (hp + 1)])

        tmps = {}

        def mul(hp):
            bch = bchs["t"]
            tmp = small_p.tile([128, 512], dt.bfloat16, tag=f"tmp{hp}", name="tmp")
            nc.vector.tensor_mul(
                tmp[0:64, :], po[2 * hp][0:64, :], bch[0:64, 512 * hp : 512 * (hp + 1)])
            nc.vector.tensor_mul(
                tmp[64:128, :], po[2 * hp + 1][0:64, :],
                bch[64:128, 512 * hp : 512 * (hp + 1)])
            tmps[hp] = tmp

        def add(hp):
            cs0 = 512 * c
            if c == NCH - 1:
                # tail: 128-col pieces so each final out-projection tile can
                # start as soon as its columns are ready
                for p4 in range(4):
                    nc.vector.tensor_scalar_add(
                        pairt[hp][:, cs0 + 128 * p4 : cs0 + 128 * (p4 + 1)],
                        tmps[hp][:, 128 * p4 : 128 * (p4 + 1)],
                        bv_sb[:, hp : hp + 1])
            else:
                nc.vector.tensor_scalar_add(
                    pairt[hp][:, cs0 : cs0 + 512], tmps[hp][:],
                    bv_sb[:, hp : hp + 1]
                )

        units = []
        for hp in hps:
            units.append(lambda hp=hp: stage_ln(hp, 0))
            units.append(lambda hp=hp: stage_ln(hp, 1))
        units.append(rcexp)
        units.append(bcast_stage)
        for hp in hps:
            units.append(lambda hp=hp: mul(hp))
            units.append(lambda hp=hp: add(hp))
        return units


# ----------------------------------------------------------------------
# ----------------------------------------------------------------------
_PROGRAM = None


def _get_program():
    global _PROGRAM
    if _PROGRAM is None:
        _PROGRAM = _build_program()
    return _PROGRAM


def _make_in_maps(inputs, W_in, b_in, W_out, b_out):
    import ml_dtypes

    in_maps = []
    scale = 1.0 / np.sqrt(np.float32(HD))
    kr = np.arange(128)[:, None]
    qc = np.arange(128)[None, :]
    trimask = np.where(qc >= kr, 1.0, 0.0).astype(ml_dtypes.bfloat16)
    sel = np.zeros((128, 128), dtype=np.float32)
    sel[0, 0:64] = 1.0
    sel[32, 64:128] = 1.0
    sel[64, 0:64] = 1.0
    sel[96, 64:128] = 1.0
    for core in range(NC):
        b, g = divmod(core, 4)
        r = slice(256 * g, 256 * (g + 1))
        wq = W_in[0:E][r] * scale
        wk = W_in[E : 2 * E][r]
        wv = W_in[2 * E : 3 * E][r]
        xT = np.ascontiguousarray(inputs[b].T).astype(ml_dtypes.bfloat16)
        wqkT = np.ascontiguousarray(np.concatenate([wq, wk], axis=0).T).astype(ml_dtypes.bfloat16)
        wvT = np.ascontiguousarray(wv.T).astype(ml_dtypes.bfloat16)
        bqk = np.concatenate([b_in[0:E][r] * scale, b_in[E : 2 * E][r]])
        bv = np.ascontiguousarray(b_in[2 * E : 3 * E][r])
        wo = np.ascontiguousarray(W_out[:, r].T).astype(ml_dtypes.bfloat16)
        in_maps.append(
            {
                "xT": xT,
                "wqkT": wqkT,
                "wvT": wvT,
                "bqk": bqk.astype(np.float32),
                "bv": bv.astype(np.float32),
                "wo": wo,
                "trimask": trimask,
                "sel": sel,
            }
        )
    return in_maps


def run_spmd(inputs, W_in, b_in, W_out, b_out, trace=False, **kw):
    nc = _get_program()
    in_maps = _make_in_maps(inputs, W_in, b_in, W_out, b_out)
    bkr = run_bass_kernel_spmd(nc, in_maps, list(range(NC)), trace=trace, **kw)
    parts = [np.asarray(bkr.results[i]["y"], dtype=np.float32) for i in range(NC)]
    out = np.stack(
        [
            parts[0] + parts[1] + parts[2] + parts[3],
            parts[4] + parts[5] + parts[6] + parts[7],
        ]
    )
    out = out + b_out[None, None, :]
    return out.astype(np.float32), bkr


def kernel(inputs, W_in, b_in, W_out, b_out):
    out, _ = run_spmd(
        np.asarray(inputs, dtype=np.float32),
        np.asarray(W_in, dtype=np.float32),
        np.asarray(b_in, dtype=np.float32),
        np.asarray(W_out, dtype=np.float32),
        np.asarray(b_out, dtype=np.float32),
    )
    return out


# revision 14
# speedup vs baseline: 1.0168x; 1.0168x over previous
"""Trainium2 Bass kernel for causal multi-head attention (B=2, S=2048, E=1024, H=16).

Sharding: 8 cores = 2 batches x 4 head-groups (4 heads each).
Each core computes its batch's QKV for its 4 heads, causal attention, and a
partial output projection; host sums the 4 group partials per batch + b_out.

V2 changes over the 218us baseline:
- bf16 for all attention-core tensors (qkt/vones/at/mask/pairt/wo/y): halves
  LDWEIGHTS, enables DVE 2x packed modes, and removes the fp32r moving<256
  penalty so causal trims can go to 128-granularity for both heads
- scores+PV trimmed to the exact block-causal minimum (off = 128*m, both
  heads); the unwritten psum window this leaves is only ever read by the exp
  into at-columns that PV never touches
- softmax reciprocal moved back to the DVE (the scalar Ln+exp staging chain
  was gating the next phase's PE work through psum-buffer reuse at every
  hp/chunk boundary, each stall also resetting the PE p-state to 1.2GHz);
  recips+bcast+muls+adds are all deferred units woven into the next phase
- startup: x chunk-0 DMA'd in token-tile pieces, v-units run first (each
  needs only its piece), qk1/qk3 pushed into the hp0 filler weave, and bf16
  warm-up matmuls ramp the PE p-state while HBM streams.
"""
import sys

sys.path.insert(0, "/opt/trn_rl_repo")

from contextlib import ExitStack

import numpy as np

import concourse.bass as bass
import concourse.tile as tile
from concourse import bacc, mybir
from concourse.bass_utils import run_bass_kernel_spmd

dt = mybir.dt

B, S, E, H = 2, 2048, 1024, 16
HD = 64                     # head dim
HPC = 4                     # heads per core
NC = 8                      # cores
KE = E // 128               # 8 contraction k-tiles for projections
NT = S // 128               # 16 token tiles
NCH = S // 512              # 4 token chunks
FQK = 512                   # q+k features per core (4 heads * 64 * 2)
FV = 256                    # v features per core


class _Bacc(bacc.Bacc):
    """Pin the activation-table fixpoint to the combined ln+exp table so the
    Exp mix never thrashes ACT_TABLE_LOADs."""

    def insert_act_table_loads(self):
        import bass_rust as _bass_rust
        from concourse.hw_specs import get_activation_tables

        has_activation = any(
            isinstance(i, mybir.InstActivation)
            for b in self.main_func.blocks
            for i in b.instructions
        )
        if not has_activation:
            return
        tables = [
            (name, funcs if name == "natural_log_exp_and_others" else set())
            for name, funcs in get_activation_tables(self.m.arch).items()
        ]
        _bass_rust.insert_act_table_loads(self, tables)


def _build_program():
    nc = _Bacc("TRN2", target_bir_lowering=False, debug=False, num_devices=NC)

    xT_d = nc.dram_tensor("xT", [E, S], dt.bfloat16, kind="ExternalInput")
    wqkT_d = nc.dram_tensor("wqkT", [E, FQK], dt.bfloat16, kind="ExternalInput")
    wvT_d = nc.dram_tensor("wvT", [E, FV], dt.bfloat16, kind="ExternalInput")
    bqk_d = nc.dram_tensor("bqk", [FQK], dt.float32, kind="ExternalInput")
    bv_d = nc.dram_tensor("bv", [FV], dt.float32, kind="ExternalInput")
    wo_d = nc.dram_tensor("wo", [FV, E], dt.bfloat16, kind="ExternalInput")
    mask_d = nc.dram_tensor("trimask", [128, 128], dt.bfloat16, kind="ExternalInput")
    sel_d = nc.dram_tensor("sel", [128, 128], dt.float32r, kind="ExternalInput")
    y_d = nc.dram_tensor("y", [S, E], dt.bfloat16, kind="ExternalOutput")

    with TileKernel(nc) as tk:
        tk.build(xT_d, wqkT_d, wvT_d, bqk_d, bv_d, wo_d, mask_d, sel_d, y_d)
    nc.compile()
    return nc


class TileKernel:
    def __init__(self, nc):
        self.nc = nc
        self.ctx = ExitStack()
        self.tc_cm = tile.TileContext(nc)

    def __enter__(self):
        self.tc = self.tc_cm.__enter__()
        return self

    def __exit__(self, *a):
        self.ctx.close()
        return self.tc_cm.__exit__(*a)

    def build(self, xT_d, wqkT_d, wvT_d, bqk_d, bv_d, wo_d, mask_d, sel_d, y_d):
        nc, tc, ctx = self.nc, self.tc, self.ctx
        pool = lambda name, bufs, **kw: ctx.enter_context(
            tc.tile_pool(name=name, bufs=bufs, **kw)
        )

        const_p = pool("const", 1)
        xs_p = pool("xs", 2)
        qkt_p = pool("qkt", 1)
        vones_p = pool("vones", 1)
        attn_p = pool("attn", 8)
        pair_p = pool("pair", 1)
        small_p = pool("small", 2)
        y_p = pool("y", 4)
        # PSUM: ps (2 bufs x [128,1024] = 4 banks) + po (4 tags x 1 bank) = 8
        ps_p = pool("ps", 2, space="PSUM")
        po_p = pool("po", 1, space="PSUM")
        self.small_p = small_p
        self.ps_p = ps_p

        # bf16 warm-up tiles for PE p-state ramp: no DMA dependency, so the
        # dummies start immediately and run at 1 cycle/row at any p-state
        dum = const_p.tile([128, 512], dt.bfloat16, tag="dum")
        nc.vector.memset(dum[:], 0.0)

        # ---- warm the exp activation table before any DMA lands ----
        warm = const_p.tile([1, 16], dt.float32, tag="warm")
        nc.vector.memset(warm[:], 0.0)
        nc.scalar.activation(warm[:], warm[:], mybir.ActivationFunctionType.Exp)

        # selector constant (tiny, used by the normalize broadcast)
        sel_sb = const_p.tile([128, 128], dt.float32r, tag="sel")
        nc.sync.dma_start(sel_sb[:], sel_d[:])
        self.sel_sb = sel_sb

        # ---- weights/x. chunk-0 x arrives in 4 token-tile pieces on the
        # scalar queue so the v-units can start after the first ~0.5MB;
        # wqk streams on the sync queue in parallel ----
        wqk_big = const_p.tile([128, KE * FQK], dt.bfloat16, tag="wqk")
        for h in range(2):
            ks = slice(4 * h, 4 * (h + 1))
            nc.sync.dma_start(
                wqk_big[:, FQK * 4 * h : FQK * 4 * (h + 1)].rearrange(
                    "p (ke f) -> p ke f", f=FQK),
                wqkT_d[512 * h : 512 * (h + 1), :].rearrange(
                    "(ke p) f -> p ke f", p=128),
            )
        wqk_r = [wqk_big[:, FQK * ke : FQK * (ke + 1)] for ke in range(KE)]

        xs0 = xs_p.tile([128, KE * 512], dt.bfloat16, tag="xs", name="xs0")
        for h in range(2):
            nc.scalar.dma_start(
                xs0[:, 2048 * h : 2048 * (h + 1)].rearrange(
                    "p (ke f) -> p ke f", f=512),
                xT_d[512 * h : 512 * (h + 1), 0:512].rearrange(
                    "(ke p) f -> p ke f", p=128),
            )
        self.xs0 = xs0

        # v weights on the gpsimd queue (v-units need them first)
        wv_big = const_p.tile([128, KE * FV], dt.bfloat16, tag="wv")
        nc.gpsimd.dma_start(
            wv_big[:].rearrange("p (ke f) -> p ke f", f=FV),
            wvT_d[:].rearrange("(ke p) f -> p ke f", p=128),
        )
        wv_r = [wv_big[:, FV * ke : FV * (ke + 1)] for ke in range(KE)]

        bqk_sb = const_p.tile([128, 4], dt.float32, tag="bqk")
        nc.sync.dma_start(bqk_sb[:], bqk_d[:].rearrange("(f p) -> p f", p=128))
        bv_sb = const_p.tile([128, 2], dt.float32, tag="bv")
        nc.sync.dma_start(bv_sb[:], bv_d[:].rearrange("(f p) -> p f", p=128))
        mask_sb = const_p.tile([128, 128], dt.bfloat16, tag="mask")
        nc.sync.dma_start(mask_sb[:], mask_d[:])

        wo_big = const_p.tile([128, 2 * E], dt.bfloat16, tag="wo")
        nc.gpsimd.dma_start(
            wo_big[:].rearrange("p (kt f) -> p kt f", f=E),
            wo_d[:].rearrange("(kt p) f -> p kt f", p=128),
        )
        wo_r = [wo_big[:, E * kt : E * (kt + 1)] for kt in range(2)]

        # persistent reciprocal tile (f32r: feeds the selector matmul).
        # ones-init once; DVE reciprocals rewrite only rows 0/32/64/96
        onef = const_p.tile([128, 512], dt.float32, tag="onef")
        nc.vector.memset(onef[:], 1.0)
        self.rc = const_p.tile([128, 512], dt.float32r, tag="rc")
        nc.vector.tensor_copy(self.rc[:], onef[:])

        # ---- persistent activations ----
        # qkt tiles: 0: q heads 0,1 | 1: q heads 2,3 | 2: k heads 0,1 | 3: k heads 2,3
        qkt = [qkt_p.tile([128, S], dt.bfloat16, tag=f"qkt{f}", name=f"qkt{f}") for f in range(4)]
        # vones[t]: [v h0 |1| v h1 |1| v h2 |1| v h3 |1] for token tile t
        vones = [vones_p.tile([128, 4 * 65], dt.bfloat16, tag=f"v{t}", name=f"v{t}") for t in range(NT)]
        # ones columns are written once here; v_unit only writes the v parts
        ones_sb = const_p.tile([128, 1, 1], dt.bfloat16, tag="ones")
        nc.vector.memset(ones_sb[:], 1.0)
        for t in range(NT):
            v3 = vones[t][:].rearrange("p (g d) -> p g d", d=65)
            nc.vector.tensor_copy(v3[:, :, 64:65], ones_sb[:].to_broadcast((128, 4, 1)))
        # pair tiles: final normalized attn output, [head dims x 2, S]
        pairt = [pair_p.tile([128, S], dt.bfloat16, tag=f"pair{hp}", name=f"pair{hp}") for hp in range(2)]

        env = dict(
            xT_d=xT_d, wqk_r=wqk_r, wv_r=wv_r, bqk_sb=bqk_sb,
            xs_p=xs_p, ps_p=ps_p, po_p=po_p, attn_p=attn_p, small_p=small_p,
            qkt=qkt, vones=vones,
            pairt=pairt, bv_sb=bv_sb, mask_sb=mask_sb, wo_r=wo_r,
            y_p=y_p, y_d=y_d,
        )
        # startup: while x0/wqk stream from HBM, run bf16 dummy matmuls so
        # the PE p-state ramps; then the v-units (piece-sized deps) and the
        # two qk units attention-hp0 needs. qk1/qk3 weave into hp0's loop.
        env[("xs", 0)] = xs0
        pwarm = ps_p.tile([128, 1024], dt.float32, tag="ps", name="pwarm")
        for _ in range(7):
            nc.tensor.matmul(pwarm[:, 0:512], dum[:, 0:128], dum[:],
                             start=True, stop=True)
        qkv0 = self.qkv_units(0, env)
        def dummy():
            nc.tensor.matmul(pwarm[:, 0:512], dum[:, 0:128], dum[:],
                             start=True, stop=True)
        for u in qkv0[4:]:      # v units t4=0..3, dummies pad DMA waits
            u()
            dummy()
        qkv0[0]()               # qk f=0 (q heads 0,1)
        dummy()
        qkv0[2]()               # qk f=2 (k heads 0,1)
        dummy()
        startup_rest = [qkv0[1], qkv0[3]]
        # filler schedule: qkv(c+1) weaves into chunk c; out-projections are
        # pushed late so chunk 3 (the longest, scalar-bound) keeps PE work:
        # oproj(0)->c1/hp1, oproj(1)->c3/hp0, oproj(2)->c3/hp1, oproj(3)->tail
        deferred = []  # normalize units of the previous chunk's rollout
        for c in range(NCH):
            if c + 1 < NCH:
                self.x_dma(c + 1, env)
                qkv = self.qkv_units(c + 1, env)
                qk_u, v_u = qkv[:4], qkv[4:]
            else:
                qk_u, v_u = [], []
            f1 = list(v_u)
            if c == 3:
                other = self.oproj_units(1, env)
                f1 += self.oproj_units(0, env) + self.oproj_units(2, env)
            else:
                other = startup_rest + qk_u
                startup_rest = []
            deferred = self.attention_chunk(c, env, deferred, other, f1)
        for u in deferred:
            u()
        for u in self.oproj_units(NCH - 1, env, copy_eng="alt"):
            u()

    # ------------------------------------------------------------------
    def x_dma(self, c, env):
        nc = self.nc
        cs = slice(512 * c, 512 * (c + 1))
        if c == 0:
            return
        xs = env["xs_p"].tile([128, KE * 512], dt.bfloat16, tag="xs", name=f"xs{c}")
        # one descriptor-gen on the sync sequencer instead of eight
        nc.sync.dma_start(
            xs[:].rearrange("p (ke f) -> p ke f", f=512),
            env["xT_d"][:, cs].rearrange("(ke p) f -> p ke f", p=128),
        )
        env[("xs", c)] = xs

    # ------------------------------------------------------------------
    def qkv_units(self, c, env):
        nc = self.nc
        cs = slice(512 * c, 512 * (c + 1))
        wqk_r, wv_r = env["wqk_r"], env["wv_r"]
        qkt, vones = env["qkt"], env["vones"]
        bqk_sb = env["bqk_sb"]
        ps_p = env["ps_p"]
        xs = env[("xs", c)]
        xr = [xs[:, 512 * ke : 512 * (ke + 1)] for ke in range(KE)]

        def qk_unit(f):
            pq = ps_p.tile([128, 1024], dt.float32, tag="ps", name="pq")
            for ke in range(KE):
                nc.tensor.matmul(
                    pq[:, 0:512], wqk_r[ke][:, 128 * f : 128 * (f + 1)], xr[ke],
                    start=(ke == 0), stop=(ke == KE - 1),
                )
            nc.vector.tensor_scalar_add(qkt[f][:, cs], pq[:, 0:512], bqk_sb[:, f : f + 1])

        def v_unit(t4):
            t = 4 * c + t4
            pv = ps_p.tile([128, 1024], dt.float32, tag="ps", name="pv")
            for ke in range(KE):
                nc.tensor.matmul(
                    pv[:, 0:FV],
                    xr[ke][:, 128 * t4 : 128 * (t4 + 1)], wv_r[ke],
                    start=(ke == 0), stop=(ke == KE - 1),
                )
            v3 = vones[t][:].rearrange("p (g d) -> p g d", d=65)
            nc.vector.tensor_copy(
                v3[:, :, 0:64],
                pv[:, 0:FV].rearrange("p (g d) -> p g d", d=64),
            )

        units = []
        for f in range(4):
            units.append(lambda f=f: qk_unit(f))
        for t4 in range(4):
            units.append(lambda t4=t4: v_unit(t4))
        return units

    # ------------------------------------------------------------------
    def oproj_units(self, c, env, copy_eng="vector"):
        nc = self.nc
        pairt, wo_r, ps_p, y_p, y_d = (
            env["pairt"], env["wo_r"], env["ps_p"], env["y_p"], env["y_d"])
        units = []

        def unit(t4):
            t = 4 * c + t4
            ysb = y_p.tile([128, E], dt.bfloat16, tag="y", name="ysb")
            py = ps_p.tile([128, 1024], dt.float32, tag="ps", name="py")
            for o in range(2):
                for kt in range(2):
                    nc.tensor.matmul(
                        py[:, 512 * o : 512 * (o + 1)],
                        pairt[kt][:, 128 * t : 128 * (t + 1)],
                        wo_r[kt][:, 512 * o : 512 * (o + 1)],
                        start=(kt == 0), stop=(kt == 1),
                    )
            ce = copy_eng if copy_eng != "alt" else ("scalar" if t4 % 2 else "vector")
            if ce == "scalar":
                nc.scalar.activation(ysb[:], py[:], mybir.ActivationFunctionType.Copy)
            else:
                nc.vector.tensor_copy(ysb[:], py[:])
            eng = nc.gpsimd if t % 2 == 0 else nc.sync
            eng.dma_start(y_d[128 * t : 128 * (t + 1), :], ysb[:])

        for t4 in range(4):
            units.append(lambda t4=t4: unit(t4))
        return units

    # ------------------------------------------------------------------
    def attention_chunk(self, c, env, pre, fillers0, f1_extra):
        """Attention for both head pairs of chunk c. Returns the deferred
        normalize units of the chunk's rollout for the caller to weave into
        the next phase. `pre` holds the previous chunk's normalize units:
        they read the po banks this chunk's first start=True PV reclaims,
        and the bcast matmul sits behind that PV in the PE FIFO, so every
        pre unit must be emitted before the first PV pop (j == PV_DEPTH)."""
        nc = self.nc
        qkt, vones = env["qkt"], env["vones"]
        ps_p, po_p, attn_p = env["ps_p"], env["po_p"], env["attn_p"]
        mask_sb = env["mask_sb"]
        nj = 4 * c + 4
        # po[2*hp + h_idx]: [65, 512] accumulator per head
        po = [po_p.tile([65, 512], dt.float32, tag=f"po{i}", name=f"po{i}")
              for i in range(4)]

        def emit_pv(hp, j, off, at):
            for h_idx in range(2):
                slot = 2 * hp + h_idx
                nc.tensor.matmul(
                    po[slot][:, off:512],
                    vones[j][:, 65 * slot : 65 * slot + 65],
                    at[:, 512 * h_idx + off : 512 * (h_idx + 1)],
                    start=(j == 0), stop=(j == nj - 1),
                    skip_group_check=True,
                )

        last = c == NCH - 1
        PV_DEPTH = 6
        part2_hp0 = None
        for hp in range(2):
            if hp == 0:
                p_units, fillers = list(pre), fillers0
            else:
                p_units, fillers = (part2_hp0 or []), f1_extra
            pre_start = min(2, nj - 1)
            slots_pre = max(1, min(PV_DEPTH, nj) - pre_start)
            quota = (len(p_units) + slots_pre - 1) // slots_pre if p_units else 0
            nfill = len(fillers)
            # reserve a couple of PE-heavy fillers for after the drain so
            # the PE stays warm across the chunk boundary while the last
            # exps retire
            nfill_w = max(0, nfill - 2) if hp == 1 else nfill
            emitted = 0
            pending = []
            for j in range(nj):
                ps = ps_p.tile([128, 1024], dt.float32, tag="ps", name="ps")
                at = attn_p.tile([128, 1024], dt.bfloat16, tag="attn", name="at")
                m = j - 4 * c
                off = 128 * m if 1 <= m <= 3 else 0
                for h_idx in range(2):
                    # bf16 runs 1 cycle/row at any moving size, so both
                    # heads trim to the exact causal boundary. The psum
                    # window [512, 512+off) this leaves unwritten holds
                    # stale-but-finite data; its exp lands in at-columns
                    # PV never reads.
                    r0 = 64 * h_idx
                    nc.tensor.matmul(
                        ps[:, 512 * h_idx + off : 512 * (h_idx + 1)],
                        qkt[2 + hp][r0 : r0 + 64, 128 * j : 128 * (j + 1)],
                        qkt[hp][r0 : r0 + 64, 512 * c + off : 512 * (c + 1)],
                        start=True, stop=True,
                    )
                nc.scalar.activation(
                    at[:, off:1024], ps[:, off:1024],
                    mybir.ActivationFunctionType.Exp)
                if m >= 0:
                    # causal mask: zero the upper triangle of the diagonal
                    # block, both heads in one strided op (bf16: DVE 2x)
                    av = at[:].rearrange("p (h q) -> p h q", h=2)
                    dg = av[:, :, 128 * m : 128 * (m + 1)]
                    mv = (mask_sb[:].rearrange("a (o n) -> a o n", o=1)
                          .to_broadcast((128, 2, 128)))
                    nc.vector.tensor_mul(dg, dg, mv)
                pending.append((j, off, at))
                if j >= pre_start:
                    for _ in range(quota):
                        if p_units:
                            p_units.pop(0)()
                if len(pending) > PV_DEPTH:
                    emit_pv(hp, *pending.pop(0))
                while emitted < nfill_w and emitted * nj < (j + 1) * nfill:
                    fillers[emitted]()
                    emitted += 1
            for u in p_units:
                u()
            for p in pending:
                emit_pv(hp, *p)
            while emitted < nfill:
                fillers[emitted]()
                emitted += 1
            if last and hp == 0:
                part2_hp0 = self.rollout(c, env, po, [0])
        if last:
            return self.rollout(c, env, po, [1])
        return self.rollout(c, env, po, [0, 1])

    # ------------------------------------------------------------------
    def rollout(self, c, env, po, hps):
        """Normalize head pairs `hps` of chunk c: pairt[hp][:,chunk] =
        po_v / denominator + bv. All units (DVE reciprocals of the psum
        denominator rows, the selector-broadcast PE outer product, the
        multiplies and the bias-add) are returned as deferred units the
        caller weaves into a later phase, so nothing gates the PE at the
        boundary."""
        nc = self.nc
        small_p, pairt, bv_sb = self.small_p, env["pairt"], env["bv_sb"]
        rc = self.rc

        def stage_ln(hp, h_idx):
            # ln(denominator) staged on the scalar engine; 1/d is then a
            # single Exp(-x). Both live in the pinned ln+exp activation
            # table, so no ACT_TABLE_LOAD thrash — a direct Reciprocal
            # would swap tables (1.3us) twice per chunk, and the DVE
            # reciprocal is an iterative op (~3.4us per [1,512] row).
            ln = mybir.ActivationFunctionType.Ln
            nc.scalar.activation(
                rc[64 * hp + 32 * h_idx : 64 * hp + 32 * h_idx + 1, :],
                po[2 * hp + h_idx][64:65, :], ln)

        def rcexp():
            lo, hi = 64 * hps[0], 64 * hps[-1] + 33
            nc.scalar.activation(
                rc[lo:hi, :], rc[lo:hi, :], mybir.ActivationFunctionType.Exp,
                scale=-1.0)

        bchs = {}

        def bcast_stage():
            bch_ps = self.ps_p.tile([128, 1024], dt.float32, tag="ps", name="bch_ps")
            for hp in hps:
                nc.tensor.matmul(
                    bch_ps[:, 512 * hp : 512 * (hp + 1)],
                    self.sel_sb[64 * hp : 64 * hp + 64, :],
                    rc[64 * hp : 64 * hp + 64, :],
                    start=True, stop=True,
                )
            bchs["t"] = small_p.tile([128, 1024], dt.float32, tag="bch", name="bchs")
            if len(hps) == 2:
                nc.vector.tensor_copy(bchs["t"][:], bch_ps[:])
            else:
                hp = hps[0]
                nc.vector.tensor_copy(
                    bchs["t"][:, 512 * hp : 512 * (hp + 1)],
                    bch_ps[:, 512 * hp : 512 * (hp + 1)])

        tmps = {}

        def mul(hp):
            bch = bchs["t"]
            tmp = small_p.tile([128, 512], dt.bfloat16, tag=f"tmp{hp}", name="tmp")
            nc.vector.tensor_mul(
                tmp[0:64, :], po[2 * hp][0:64, :], bch[0:64, 512 * hp : 512 * (hp + 1)])
            nc.vector.tensor_mul(
                tmp[64:128, :], po[2 * hp + 1][0:64, :],
                bch[64:128, 512 * hp : 512 * (hp + 1)])
            tmps[hp] = tmp

        def add(hp):
            cs0 = 512 * c
            if c == NCH - 1:
                # tail: 128-col pieces so each final out-projection tile can
                # start as soon as its columns are ready
                for p4 in range(4):
                    nc.vector.tensor_scalar_add(
                        pairt[hp][:, cs0 + 128 * p4 : cs0 + 128 * (p4 + 1)],
                        tmps[hp][:, 128 * p4 : 128 * (p4 + 1)],
                        bv_sb[:, hp : hp + 1])
            else:
                nc.vector.tensor_scalar_add(
                    pairt[hp][:, cs0 : cs0 + 512], tmps[hp][:],
                    bv_sb[:, hp : hp + 1]
                )

        units = []
        for hp in hps:
            units.append(lambda hp=hp: stage_ln(hp, 0))
            units.append(lambda hp=hp: stage_ln(hp, 1))
        units.append(rcexp)
        units.append(bcast_stage)
        for hp in hps:
            units.append(lambda hp=hp: mul(hp))
            units.append(lambda hp=hp: add(hp))
        return units


# ----------------------------------------------------------------------
# ----------------------------------------------------------------------
_PROGRAM = None


def _get_program():
    global _PROGRAM
    if _PROGRAM is None:
        _PROGRAM = _build_program()
    return _PROGRAM


def _make_in_maps(inputs, W_in, b_in, W_out, b_out):
    import ml_dtypes

    in_maps = []
    scale = 1.0 / np.sqrt(np.float32(HD))
    kr = np.arange(128)[:, None]
    qc = np.arange(128)[None, :]
    trimask = np.where(qc >= kr, 1.0, 0.0).astype(ml_dtypes.bfloat16)
    sel = np.zeros((128, 128), dtype=np.float32)
    sel[0, 0:64] = 1.0
    sel[32, 64:128] = 1.0
    sel[64, 0:64] = 1.0
    sel[96, 64:128] = 1.0
    for core in range(NC):
        b, g = divmod(core, 4)
        r = slice(256 * g, 256 * (g + 1))
        wq = W_in[0:E][r] * scale
        wk = W_in[E : 2 * E][r]
        wv = W_in[2 * E : 3 * E][r]
        xT = np.ascontiguousarray(inputs[b].T).astype(ml_dtypes.bfloat16)
        wqkT = np.ascontiguousarray(np.concatenate([wq, wk], axis=0).T).astype(ml_dtypes.bfloat16)
        wvT = np.ascontiguousarray(wv.T).astype(ml_dtypes.bfloat16)
        bqk = np.concatenate([b_in[0:E][r] * scale, b_in[E : 2 * E][r]])
        bv = np.ascontiguousarray(b_in[2 * E : 3 * E][r])
        wo = np.ascontiguousarray(W_out[:, r].T).astype(ml_dtypes.bfloat16)
        in_maps.append(
            {
                "xT": xT,
                "wqkT": wqkT,
                "wvT": wvT,
                "bqk": bqk.astype(np.float32),
                "bv": bv.astype(np.float32),
                "wo": wo,
                "trimask": trimask,
                "sel": sel,
            }
        )
    return in_maps


def run_spmd(inputs, W_in, b_in, W_out, b_out, trace=False, **kw):
    nc = _get_program()
    in_maps = _make_in_maps(inputs, W_in, b_in, W_out, b_out)
    bkr = run_bass_kernel_spmd(nc, in_maps, list(range(NC)), trace=trace, **kw)
    parts = [np.asarray(bkr.results[i]["y"], dtype=np.float32) for i in range(NC)]
    out = np.stack(
        [
            parts[0] + parts[1] + parts[2] + parts[3],
            parts[4] + parts[5] + parts[6] + parts[7],
        ]
    )
    out = out + b_out[None, None, :]
    return out.astype(np.float32), bkr


def kernel(inputs, W_in, b_in, W_out, b_out):
    out, _ = run_spmd(
        np.asarray(inputs, dtype=np.float32),
        np.asarray(W_in, dtype=np.float32),
        np.asarray(b_in, dtype=np.float32),
        np.asarray(W_out, dtype=np.float32),
        np.asarray(b_out, dtype=np.float32),
    )
    return out
